# revision 30
# baseline (speedup 1.0000x reference)
# Trainium2 Bass kernel for nn_Attention_88029649699625 (gated multi-head
# attention block with residual-gate MLP).
#
# Sharding: collective-free split over (batch, query-half). Core c = (b, j)
# with b = c // 2, j = c % 2 handles all 16 heads for query tokens
# [j*1024, (j+1)*1024) of batch b. k/v projections for batch b are computed
# on both cores sharing that batch (extra FLOPs, far cheaper than any
# on-chip collective). Each core's output is a disjoint token slice; the
# host concatenates.
#
# v2 highlights vs the original baseline:
#  - scores matmuls for the two heads of a head-pair are interleaved so the
#    PE row-tiles (64x128 mode, tiles T0/T8) run concurrently -> 2x scores.
#  - softmax exp runs as a single DVE op per tile (Schraudolph-style affine
#    -> int8 bits -> fp8e4 view) for most k-batches, with a share on ACT
#    (true exp -> fp8 out) so neither engine is the wall.
#  - the attention mix runs in fp8 DoubleRow (2 k-tiles per matmul -> 2x).
#    v slabs hold raw v + v_bias (vb_bc added at the drain); the vs gate is
#    applied post-normalization as a per-partition scalar, so no bias
#    matmuls and no vs broadcast are needed.
#  - one rotating 4-slot PSUM pool ([P, 2, CH] slots) serves all phases.

import os
import numpy as np
import ml_dtypes

BF16 = ml_dtypes.bfloat16
FP8 = ml_dtypes.float8_e4m3

# Problem dims (hardcoded per the harness contract)
SEQ, BATCH, NHID, HEADS, DHEAD = 2048, 4, 1024, 16, 64
NCORES = 8
P = 128

# Schraudolph exp constants (fp8e4m3 bits): value 2^(e-7)*(1+m/8), bits
# i8 = 8*log2(v) + 56. For v = exp(s/8): i8 = s*log2e + (56 - C8) with C8
# tuned on CPU to balance the linear-mantissa interpolation error.
SCHR_A = float(np.log2(np.e))          # 1.4426950
SCHR_B = 56.0 - 0.35                   # C8_round = 0.35


class Cfg:
    def __init__(self, seq=SEQ, batch=BATCH, nhid=NHID, dhead=DHEAD):
        self.seq = seq
        self.batch = batch
        self.nhid = nhid
        self.dhead = dhead
        self.heads = nhid // dhead
        self.tq = seq * batch // NCORES   # query tokens per core
        self.tk = seq                     # kv tokens per core (one batch)
        self.et = nhid // P               # e-tiles (also head-pairs)
        self.it = nhid // P               # i-tiles (contraction)
        self.kt = self.tk // P            # k-token tiles
        self.vq = 2 * nhid                # overparam width
        self.ch = min(512, self.tq)       # token chunk (psum free dim)
        assert self.tq % self.ch == 0 and self.tk % self.ch == 0
        self.nqch = self.tq // self.ch
        assert self.dhead == 64, "head packing assumes d=64 (2 heads / 128 partitions)"


FULL = Cfg()


def build(cfg=FULL, phases="all", repeat=1):
    """Build the per-core Bass program (SPMD: same program, per-core data).
    repeat>1 emits the whole body R times (for slope-based timing: the RPC
    floor is constant, so kernel time = (T_R - T_1) / (R - 1)).
    phases truncates the build for phase-level timing: "proj" stops after
    projections, "attn" stops after attention (output is garbage)."""
    import concourse.bass as bass
    import concourse.mybir as mybir
    import concourse.tile as tile
    from concourse import bacc

    bf = mybir.dt.bfloat16
    f32 = mybir.dt.float32
    fp8 = mybir.dt.float8e4
    i8 = mybir.dt.int8
    AF = mybir.ActivationFunctionType
    OP = mybir.AluOpType
    PM = mybir.MatmulPerfMode

    ET, IT, KT, CH, TQ, TK, NH, VQ = (
        cfg.et, cfg.it, cfg.kt, cfg.ch, cfg.tq, cfg.tk, cfg.nhid, cfg.vq)
    NKCH = TK // CH          # k-proj token chunks
    NECH = NH // CH          # e chunks (v-proj)
    NFCH = VQ // CH          # vq chunks
    H = cfg.heads
    NPAIR = KT // 2          # k-tile pairs for DoubleRow mix

    ACT_EXP_EVERY = int(os.environ.get("K_ACT_EXP_EVERY", "3"))  # 1 of every N
    DVE_EXP = os.environ.get("K_DVE_EXP", "1") == "1"
    KV_FP8 = os.environ.get("K_KV_FP8", "1") == "1"  # k/v proj in fp8 DoubleRow

    nc = bacc.Bacc(None)

    # ---- DRAM I/O (per-core, host pre-laid-out; see prep_core_inputs) ----
    kvdt = fp8 if KV_FP8 else bf
    d_xq = nc.dram_tensor("xq", [P, IT, TQ], bf, kind="ExternalInput")
    d_xk = nc.dram_tensor("xk", [P, IT, TK], kvdt, kind="ExternalInput")
    d_xv = nc.dram_tensor("xv", [P, IT, TK], kvdt, kind="ExternalInput")
    d_qw = nc.dram_tensor("qw", [P, ET, IT, P], bf, kind="ExternalInput")
    # kw: DoubleRow pairs along the contraction i-tiles when KV_FP8
    if KV_FP8:
        d_kw = nc.dram_tensor("kw", [P, ET, IT // 2, 2, P], kvdt,
                              kind="ExternalInput")
    else:
        d_kw = nc.dram_tensor("kw", [P, ET, IT, P], kvdt, kind="ExternalInput")
    d_vw = nc.dram_tensor("vw", [P, IT, NH], kvdt, kind="ExternalInput")
    d_w1 = nc.dram_tensor("w1", [P, ET, ET, P], bf, kind="ExternalInput")
    d_w2 = nc.dram_tensor("w2", [P, ET, ET, P], bf, kind="ExternalInput")
    d_vqw = nc.dram_tensor("vqw", [P, NFCH, IT, CH], bf, kind="ExternalInput")
    # per-partition vectors [P, ET] (pp layout: x.reshape(ET, P).T)
    d_qsp = nc.dram_tensor("qsp", [P, ET], f32, kind="ExternalInput")
    d_ksp = nc.dram_tensor("ksp", [P, ET], f32, kind="ExternalInput")
    d_rgp = nc.dram_tensor("rgp", [P, ET], f32, kind="ExternalInput")
    d_qb = nc.dram_tensor("qb", [P, ET], f32, kind="ExternalInput")
    d_kb = nc.dram_tensor("kb", [P, ET], f32, kind="ExternalInput")
    d_rb = nc.dram_tensor("rb", [P, ET], f32, kind="ExternalInput")
    d_vsp = nc.dram_tensor("vsp", [P, ET], f32, kind="ExternalInput")
    # rows / tables
    d_vbbc = nc.dram_tensor("vbbc", [P, NH], bf, kind="ExternalInput")
    d_vqbr = nc.dram_tensor("vqbr", [1, VQ], f32, kind="ExternalInput")
    # scratch for transposing the on-device vs row into per-partition layout
    d_vscr = nc.dram_tensor("vscr", [ET, P], f32, kind="Internal")
    d_out = nc.dram_tensor("out", [P, ET, TQ], f32, kind="ExternalOutput")

    from contextlib import ExitStack

    with tile.TileContext(nc) as tc:
      for _rep in range(repeat):
        with ExitStack() as stk:
            cp = stk.enter_context(tc.tile_pool(name="const", bufs=1))
            bigp = stk.enter_context(tc.tile_pool(name="big", bufs=1))
            sp = stk.enter_context(tc.tile_pool(name="stage", bufs=2))
            pp = stk.enter_context(tc.tile_pool(name="ps", bufs=4, space="PSUM"))

            _psn = [0]

            def psum(shape, tag="ps"):
                # single rotating tag: slot sized by the largest tile
                # ([P, 2, CH] f32 = 2 banks), 4 slots = all 8 PSUM banks
                _psn[0] += 1
                return pp.tile(shape, f32, tag="ps", name=f"ps{_psn[0]}")

            # ---- persistent small constants ----
            rb = cp.tile([P, ET], f32); nc.sync.dma_start(rb[:], d_rb[:])
            s_qs = cp.tile([P, ET], f32)
            s_ks = cp.tile([P, ET], f32)
            s_rg = cp.tile([P, ET], f32)
            qb_eff = cp.tile([P, ET], f32)
            kb_eff = cp.tile([P, ET], f32)
            rb_eff = cp.tile([P, ET], f32)
            vs_pp = cp.tile([P, ET], f32)
            vb_bc = cp.tile([P, NH], bf); nc.sync.dma_start(vb_bc[:], d_vbbc[:])
            ident64 = cp.tile([64, 64], bf)
            from concourse.masks import make_identity
            make_identity(nc, ident64[:])
            # E64: row 64 = ones, rest 0 — broadcasts the reciprocal row via
            # a (64,64)-tile matmul reading partitions 64:128 (same PE mode
            # as the B-half identity move)
            E64 = cp.tile([P, 64], bf)
            nc.vector.memset(E64[:], 0.0)
            nc.vector.memset(E64[64:65, :], 1.0)
            # persistent reciprocal rows (partitions 65:128 stay 1.0 so the
            # zero-weight junk rows of the broadcast matmul never see NaN)
            recA_t = cp.tile([P, CH], bf)
            nc.vector.memset(recA_t[:], 1.0)
            recB_t = cp.tile([P, CH], bf)
            nc.vector.memset(recB_t[:], 1.0)

            # ---- persistent big activations ----
            kT = bigp.tile([P, ET, TK], bf)        # gated k projection, [e, t]
            qT = bigp.tile([P, ET, TQ], bf)        # gated q projection, [e, t]
            # v slabs for DoubleRow mix: [tok, pair, j, head, d + ones]
            v_st = bigp.tile([P, NPAIR, 2, H, 65], fp8)
            mixT = bigp.tile([P, ET, TQ], bf)      # normalized mix, [e, t]
            nc.vector.memset(v_st[:, :, :, :, 64:65], 1.0)

            # ======== phase 0 + projections (one scope; DMA-ordered) ========
            # DMA order: q-proj inputs first (critical path), then k-proj
            # th0; the 4MB vq weight and the vq/vs computation sit between
            # q-proj and k-proj so their DMA/compute overlap the pipeline.
            TKH = TK // 2
            with tc.tile_pool(name="xw", bufs=2) as xw, \
                 tc.tile_pool(name="pw", bufs=2) as pw, \
                 tc.tile_pool(name="ph0", bufs=1) as p0, \
                 tc.tile_pool(name="vqw", bufs=1) as vqp:
                xq = xw.tile([P, IT, TQ], bf, tag="xw")
                nc.sync.dma_start(xq[:], d_xq[:])
                qw = pw.tile([P, ET, IT, P], bf, tag="pw")
                nc.sync.dma_start(qw[:], d_qw[:])
                if KV_FP8:
                    kw = pw.tile([P, ET, IT // 2, 2, P], kvdt, tag="pw")
                else:
                    kw = pw.tile([P, ET, IT, P], kvdt, tag="pw")
                nc.scalar.dma_start(kw[:], d_kw[:])
                xk0 = xw.tile([P, IT, TKH], kvdt, tag="xw")
                nc.scalar.dma_start(xk0[:], d_xk[:, :, 0:TKH])

                qsp = p0.tile([P, ET], f32); nc.sync.dma_start(qsp[:], d_qsp[:])
                ksp = p0.tile([P, ET], f32); nc.sync.dma_start(ksp[:], d_ksp[:])
                rgp = p0.tile([P, ET], f32); nc.sync.dma_start(rgp[:], d_rgp[:])
                qb = p0.tile([P, ET], f32); nc.sync.dma_start(qb[:], d_qb[:])
                kb = p0.tile([P, ET], f32); nc.sync.dma_start(kb[:], d_kb[:])
                vsp = p0.tile([P, ET], f32); nc.sync.dma_start(vsp[:], d_vsp[:])
                vqbr = p0.tile([1, VQ], f32); nc.sync.dma_start(vqbr[:], d_vqbr[:])

                nc.scalar.activation(s_qs[:], qsp[:], AF.Sigmoid)
                nc.scalar.activation(s_ks[:], ksp[:], AF.Sigmoid)
                nc.scalar.activation(s_rg[:], rgp[:], AF.Sigmoid)
                vs0f = p0.tile([P, ET], f32)
                nc.scalar.activation(vs0f[:], vsp[:], AF.Sigmoid)
                vs0b = p0.tile([P, ET], bf)
                nc.vector.tensor_copy(vs0b[:], vs0f[:])
                nc.vector.tensor_tensor(qb_eff[:], qb[:], s_qs[:], op=OP.mult)
                nc.vector.tensor_tensor(kb_eff[:], kb[:], s_ks[:], op=OP.mult)
                nc.vector.tensor_scalar(rb_eff[:], rb[:], 1.702, None, op0=OP.mult)

                # ---- q projection ----
                for et in range(ET):
                    for tch in range(cfg.nqch):
                        tsl = slice(tch * CH, (tch + 1) * CH)
                        ps = psum([P, CH])
                        for it in range(IT):
                            nc.tensor.matmul(ps[:], qw[:, et, it], xq[:, it, tsl],
                                             start=(it == 0), stop=(it == IT - 1))
                        nc.vector.tensor_scalar(qT[:, et, tsl], ps[:],
                                                s_qs[:, et:et + 1],
                                                qb_eff[:, et:et + 1],
                                                op0=OP.mult, op1=OP.add)

                # ---- vs gate overparam (same 128x128 mode as projections;
                # vq weight DMA overlaps with k-proj inputs already resident)
                cf_sb = p0.tile([1, VQ], f32)
                for fch in range(NFCH):
                    vqw_c = vqp.tile([P, IT, CH], bf, tag="vq")
                    nc.scalar.dma_start(vqw_c[:], d_vqw[:, fch])
                    pc = psum([1, CH])
                    for it in range(IT):
                        nc.tensor.matmul(pc[:], vs0b[:, it:it + 1], vqw_c[:, it],
                                         start=(it == 0), stop=(it == IT - 1))
                    fsl = slice(fch * CH, (fch + 1) * CH)
                    nc.vector.tensor_tensor(cf_sb[:, fsl], pc[:], vqbr[:, fsl],
                                            op=OP.add)
                # vs = sigmoid(f) * tanh(c); c = cf[:NH], f = cf[NH:]
                tanh_c = p0.tile([1, NH], f32)
                nc.scalar.activation(tanh_c[:], cf_sb[:, 0:NH], AF.Tanh)
                vs_row = p0.tile([1, NH], f32)
                nc.scalar.activation(vs_row[:], cf_sb[:, NH:VQ], AF.Sigmoid)
                nc.vector.tensor_tensor(vs_row[:], vs_row[:], tanh_c[:], op=OP.mult)
                # transpose the row into per-partition [P, ET] via a DRAM
                # round-trip (feature e*128+p -> partition p, column e)
                nc.sync.dma_start(d_vscr[:, :], vs_row[:])
                for e in range(ET):
                    nc.sync.dma_start(vs_pp[:, e:e + 1], d_vscr[e:e + 1, :])

                # ---- k projection ----
                for th in range(2):
                    if th == 0:
                        xk_h = xk0
                    else:
                        xk_h = xw.tile([P, IT, TKH], kvdt, tag="xw")
                        nc.scalar.dma_start(xk_h[:],
                                            d_xk[:, :, th * TKH:(th + 1) * TKH])
                    for tcl in range(NKCH // 2):
                        lsl = slice(tcl * CH, (tcl + 1) * CH)
                        tsl = slice(th * TKH + tcl * CH, th * TKH + (tcl + 1) * CH)
                        for et in range(ET):
                            ps = psum([P, CH])
                            if KV_FP8:
                                for ip in range(IT // 2):
                                    nc.tensor.matmul(
                                        ps[:], kw[:, et, ip],
                                        xk_h[:, 2 * ip:2 * ip + 2, lsl],
                                        start=(ip == 0), stop=(ip == IT // 2 - 1),
                                        perf_mode=PM.DoubleRow)
                            else:
                                for it in range(IT):
                                    nc.tensor.matmul(ps[:], kw[:, et, it],
                                                     xk_h[:, it, lsl],
                                                     start=(it == 0),
                                                     stop=(it == IT - 1))
                            nc.vector.tensor_scalar(kT[:, et, tsl], ps[:],
                                                    s_ks[:, et:et + 1],
                                                    kb_eff[:, et:et + 1],
                                                    op0=OP.mult, op1=OP.add)

                vw = pw.tile([P, IT, NH], kvdt, tag="pw")
                nc.scalar.dma_start(vw[:], d_vw[:])
                # v token-major raw slabs + v_b (vs gate applied post-norm)
                for th in range(2):
                    xv_h = xw.tile([P, IT, TKH], kvdt, tag="xw")
                    nc.scalar.dma_start(xv_h[:], d_xv[:, :, th * TKH:(th + 1) * TKH])
                    for ttl in range(KT // 2):
                        tt = th * (KT // 2) + ttl
                        ltsl = slice(ttl * P, (ttl + 1) * P)
                        for ech in range(NECH):
                            esl = slice(ech * CH, (ech + 1) * CH)
                            ps = psum([P, CH])
                            if KV_FP8:
                                for ip in range(IT // 2):
                                    nc.tensor.matmul(
                                        ps[:], xv_h[:, 2 * ip:2 * ip + 2, ltsl],
                                        vw[:, 2 * ip:2 * ip + 2, esl],
                                        start=(ip == 0), stop=(ip == IT // 2 - 1),
                                        perf_mode=PM.DoubleRow)
                            else:
                                for it in range(IT):
                                    nc.tensor.matmul(ps[:], xv_h[:, it, ltsl],
                                                     vw[:, it, esl],
                                                     start=(it == 0),
                                                     stop=(it == IT - 1))
                            hsl = slice(ech * (CH // 64), (ech + 1) * (CH // 64))
                            nc.vector.tensor_tensor(
                                v_st[:, tt // 2, tt % 2, hsl, 0:64], ps[:],
                                vb_bc[:, esl], op=OP.add)

            if phases == "proj":
                dump = sp.tile([P, CH], f32, tag="sg")
                nc.vector.tensor_copy(dump[:], kT[:, 0, 0:CH])
                nc.sync.dma_start(d_out[:, 0, 0:CH], dump[:])
                continue

            # ======== attention (per q-chunk, per head-pair) ========
            wz = stk.enter_context(tc.tile_pool(name="wz", bufs=2))
            ep = stk.enter_context(tc.tile_pool(name="exp", bufs=1))
            w1 = wz.tile([P, ET, ET, P], bf, tag="wz")
            nc.sync.dma_start(w1[:], d_w1[:])
            w2 = wz.tile([P, ET, ET, P], bf, tag="wz")
            nc.sync.dma_start(w2[:], d_w2[:])

            # one MLP output chain; interleaved into the NEXT q-chunk's
            # attention so ready-to-run matmuls fill the PE's DVE-wait
            # bubbles (the last q-chunk's chains run as the tail)
            def mlp_ot(qch_i, ot):
                qsl_i = slice(qch_i * CH, (qch_i + 1) * CH)
                pz = psum([P, CH])
                for et in range(ET):
                    nc.tensor.matmul(pz[:], w1[:, ot, et], mixT[:, et, qsl_i],
                                     start=(et == 0), stop=False)
                for et in range(ET):
                    nc.tensor.matmul(pz[:], w2[:, ot, et], qT[:, et, qsl_i],
                                     start=False, stop=(et == ET - 1))
                sg = sp.tile([P, CH], f32, tag="sg")
                nc.scalar.activation(sg[:], pz[:], AF.Sigmoid, scale=1.702,
                                     bias=rb_eff[:, ot:ot + 1])
                rr = sp.tile([P, CH], f32, tag="rr")
                nc.vector.scalar_tensor_tensor(rr[:], pz[:], rb[:, ot:ot + 1],
                                               sg[:], op0=OP.add, op1=OP.mult)
                oo = sp.tile([P, CH], f32, tag="oo")
                nc.vector.scalar_tensor_tensor(oo[:], mixT[:, ot, qsl_i],
                                               s_rg[:, ot:ot + 1], rr[:],
                                               op0=OP.mult, op1=OP.add)
                nc.sync.dma_start(d_out[:, ot, qsl_i], oo[:])

            for qch in range(cfg.nqch):
                qsl = slice(qch * CH, (qch + 1) * CH)
                stg_prev = None
                for hp in range(ET + 1):
                    # -- deferred B-half move of the previous head-pair: runs
                    # after this hp's scores are queued elsewhere... placed
                    # here so the PE never waits on the DVE normalize chain
                    if hp < ET:
                        expA = ep.tile([P, KT, CH], fp8, tag="expA")
                        expB = ep.tile([P, KT, CH], fp8, tag="expB")
                        expA8 = expA.bitcast(i8)
                        expB8 = expB.bitcast(i8)
                        # -- scores (64x128 row-tiled pairs) + exp --
                        for kb in range(KT // 2):
                            k0 = 2 * kb
                            psA = psum([P, 2, CH])
                            psB = psum([P, 2, CH])
                            for u in range(2):
                                kt = k0 + u
                                ksl = slice(kt * P, (kt + 1) * P)
                                nc.tensor.matmul(psA[:, u], kT[0:64, hp, ksl],
                                                 qT[0:64, hp, qsl],
                                                 start=True, stop=True)
                                nc.tensor.matmul(psB[:, u], kT[64:128, hp, ksl],
                                                 qT[64:128, hp, qsl],
                                                 start=True, stop=True)
                            esl = slice(k0, k0 + 2)
                            if DVE_EXP and (kb % ACT_EXP_EVERY !=
                                            ACT_EXP_EVERY - 1):
                                nc.vector.tensor_scalar(expA8[:, esl, :], psA[:],
                                                        SCHR_A, SCHR_B,
                                                        op0=OP.mult, op1=OP.add)
                                nc.vector.tensor_scalar(expB8[:, esl, :], psB[:],
                                                        SCHR_A, SCHR_B,
                                                        op0=OP.mult, op1=OP.add)
                            else:
                                nc.scalar.activation(expA[:, esl, :], psA[:],
                                                     AF.Exp, scale=1.0 / 8.0)
                                nc.scalar.activation(expB[:, esl, :], psB[:],
                                                     AF.Exp, scale=1.0 / 8.0)
                    # previous hp's B-half move (ident + vs), pipelined so
                    # its DVE inputs are ready by the time the PE gets here
                    if stg_prev is not None:
                        php, stg = stg_prev
                        pmv = psum([P, CH])
                        nc.tensor.matmul(pmv[64:128, :], ident64[:], stg[:],
                                         start=True, stop=True)
                        nc.vector.tensor_scalar(mixT[64:128, php, qsl],
                                                pmv[64:128, :],
                                                vs_pp[64:128, php:php + 1],
                                                None, op0=OP.mult)
                        stg_prev = None
                    if hp == ET:
                        break
                    # -- mix (fp8 DoubleRow, 2 k-tiles per matmul) --
                    pmA = psum([65, CH])
                    pmB = psum([65, CH])
                    for pm_t, hh in ((pmA, 0), (pmB, 1)):
                        h = 2 * hp + hh
                        expT = expA if hh == 0 else expB
                        for pr in range(NPAIR):
                            nc.tensor.matmul(pm_t[:], v_st[:, pr, :, h, :],
                                             expT[:, 2 * pr:2 * pr + 2, :],
                                             start=(pr == 0),
                                             stop=(pr == NPAIR - 1),
                                             perf_mode=PM.DoubleRow)
                    # -- normalize + vs gate; the rec broadcast runs as a
                    # (64,64)-tile matmul on partitions 64:128 (E64 row) --
                    with nc.allow_low_precision(
                            reason="softmax denom reciprocal in bf16: den "
                                   "~2048, 0.4% rel is within budget"):
                        nc.vector.reciprocal(recA_t[64:65, :], pmA[64:65, :])
                        nc.vector.reciprocal(recB_t[64:65, :], pmB[64:65, :])
                    pbcA = psum([64, CH])
                    nc.tensor.matmul(pbcA[:], E64[64:128, :], recA_t[64:128, :],
                                     start=True, stop=True)
                    pbcB = psum([64, CH])
                    nc.tensor.matmul(pbcB[:], E64[64:128, :], recB_t[64:128, :],
                                     start=True, stop=True)
                    rsbA = sp.tile([64, CH], f32, tag="rsbA", bufs=1)
                    nc.vector.tensor_copy(rsbA[:], pbcA[:])
                    rsbB = sp.tile([64, CH], f32, tag="rsbB", bufs=1)
                    nc.vector.tensor_copy(rsbB[:], pbcB[:])
                    # head A: mixT[0:64] = (pm * vs) * rec
                    nc.vector.scalar_tensor_tensor(
                        mixT[0:64, hp, qsl], pmA[0:64], vs_pp[0:64, hp:hp + 1],
                        rsbA[:], op0=OP.mult, op1=OP.mult)
                    # head B: normalize now, move next iteration
                    stg = sp.tile([64, CH], bf, tag="stg", bufs=2)
                    nc.vector.tensor_tensor(stg[:], pmB[0:64], rsbB[:], op=OP.mult)
                    stg_prev = (hp, stg)
                    if qch > 0 and phases != "attn":
                        mlp_ot(qch - 1, hp)

            if phases == "attn":
                dump = sp.tile([P, CH], f32, tag="sg")
                nc.vector.tensor_copy(dump[:], mixT[:, 0, 0:CH])
                nc.sync.dma_start(d_out[:, 0, 0:CH], dump[:])
                continue

            # ======== residual-gate MLP (tail: last q-chunk only; earlier
            # q-chunks were interleaved into the attention loop) ========
            # z = mix @ r_w[:, :NH].T + q @ r_w[:, NH:].T
            # out = sigmoid(r_gate) * mix + (z + r_b) * sigmoid(1.702 (z + r_b))
            for ot in range(ET):
                mlp_ot(cfg.nqch - 1, ot)

    nc.compile()
    return nc


# ---------------- host-side data prep ----------------

def _pp(x, cfg):
    return np.ascontiguousarray(
        np.asarray(x, np.float32).reshape(-1).reshape(cfg.et, P).T)


def prep_shared(cfg, inputs):
    """Weights/gates: identical for every core."""
    f32 = np.float32
    nh, it, et, vq = cfg.nhid, cfg.it, cfg.et, cfg.vq
    q_w = np.asarray(inputs["q_w"], f32)
    k_w = np.asarray(inputs["k_w"], f32)
    v_w = np.asarray(inputs["v_w"], f32)
    r_w = np.asarray(inputs["r_w"], f32)
    vq_w = np.asarray(inputs["vq_w"], f32)

    def lhsT_tiles(w, dt=BF16):  # [out, in] -> [p(i), ot, it, o]
        return np.ascontiguousarray(
            w.reshape(et, P, it, P).transpose(3, 0, 2, 1).astype(dt))

    kv_fp8 = os.environ.get("K_KV_FP8", "1") == "1"
    kvdt = FP8 if kv_fp8 else BF16
    if kv_fp8:
        # DoubleRow pairs along contraction i-tiles:
        # kw[p, ot, ip, j, o] = k_w[ot*128+o, (2*ip+j)*128+p]
        kw_prep = np.ascontiguousarray(
            k_w.reshape(et, P, it // 2, 2, P).transpose(4, 0, 2, 3, 1)
            .astype(FP8))
    else:
        kw_prep = lhsT_tiles(k_w, kvdt)
    shared = {
        "qw": lhsT_tiles(q_w),
        "kw": kw_prep,
        "vw": np.ascontiguousarray(
            v_w.T.reshape(it, P, nh).transpose(1, 0, 2).astype(kvdt)),
        "qsp": _pp(inputs["qs_p"], cfg),
        "ksp": _pp(inputs["ks_p"], cfg),
        "rgp": _pp(inputs["r_gate"], cfg),
        "qb": _pp(inputs["q_b"], cfg),
        "kb": _pp(inputs["k_b"], cfg),
        "rb": _pp(inputs["r_b"], cfg),
        "vsp": _pp(inputs["vs_p"], cfg),
        "vbbc": np.ascontiguousarray(
            np.broadcast_to(np.asarray(inputs["v_b"], f32).reshape(1, nh),
                            (P, nh)).astype(BF16)),
        "vqbr": np.asarray(inputs["vq_b"], f32).reshape(1, vq).copy(),
    }
    # w1/w2: element [p(e), ot, et, o] = r_w[ot*P+o, et*P+e_local]
    shared["w1"] = lhsT_tiles(r_w[:, :nh])
    shared["w2"] = lhsT_tiles(r_w[:, nh:])
    # vqw: [p(i), fch, it, ch] = vq_w[fch*CH + f, it*P + p]
    nfch = vq // cfg.ch
    shared["vqw"] = np.ascontiguousarray(
        vq_w.T.reshape(it, P, nfch, cfg.ch).transpose(1, 2, 0, 3).astype(BF16))
    return shared


def _tok_major(x_t_f, it, dt=BF16):
    """[tokens, feat] -> [P, it, tokens] (transposed, partition-tiled)."""
    t, f = x_t_f.shape
    return np.ascontiguousarray(
        x_t_f.T.reshape(it, P, t).transpose(1, 0, 2).astype(dt))


def prep_core_inputs(cfg, inputs, shared, core):
    b, j = core // 2, core % 2
    tq = cfg.tq
    kvdt = FP8 if os.environ.get("K_KV_FP8", "1") == "1" else BF16
    query = np.asarray(inputs["query"], np.float32)
    key = np.asarray(inputs["key"], np.float32)
    value = np.asarray(inputs["value"], np.float32)
    m = dict(shared)
    m["xq"] = _tok_major(query[j * tq:(j + 1) * tq, b, :], cfg.it)
    m["xk"] = _tok_major(key[:, b, :], cfg.it, kvdt)
    m["xv"] = _tok_major(value[:, b, :], cfg.it, kvdt)
    return m


def assemble(cfg, results):
    """Per-core outT [P, et, TQ] -> full [SEQ, BATCH, NHID] f32."""
    out = np.empty((cfg.seq, cfg.batch, cfg.nhid), np.float32)
    for c, res in enumerate(results):
        b, j = c // 2, c % 2
        o = np.asarray(res["out"], np.float32)       # [P, et, TQ]
        o = o.transpose(1, 0, 2).reshape(cfg.nhid, cfg.tq)  # [NHID, TQ]
        out[j * cfg.tq:(j + 1) * cfg.tq, b, :] = o.T
    return out


_CACHED_NC = None


def kernel(**inputs):
    global _CACHED_NC
    from concourse.bass_utils import run_bass_kernel_spmd

    cfg = FULL
    if _CACHED_NC is None:
        _CACHED_NC = build(cfg)
    nc = _CACHED_NC

    shared = prep_shared(cfg, inputs)
    in_maps = [prep_core_inputs(cfg, inputs, shared, c) for c in range(NCORES)]
    res = run_bass_kernel_spmd(nc, in_maps, list(range(NCORES)))
    return assemble(cfg, res.results)
